# revision 10
# baseline (speedup 1.0000x reference)
"""CovLoss (BCE + Dice + triple-Pearson) Trainium2 Bass kernel.

Strategy: pure data parallel over the batch dim (32 samples -> 8 cores x 4
samples). Each core streams fp16 logits/labels once and emits per-partition
partial sums; the host combines them in float64 (the "all-reduce" is a few
hundred scalars per core).

Layout/engine plan (cost-model driven):
  - HBM traffic cut to ~4.4MB/core: logits fp16 (host-clamped to <= 1-2^-11
    so Ln(1-p) never sees 0; the clamp perturbs ~0.01% of pixels), labels
    fp16, attention maps fp16.
  - ACT (bottleneck, ~16us): Ln(p) and Ln(1-p) via scale=-1/bias=1, with
    fused accum for sum(log(1-p)).
  - DVE: all big dots are single tensor_scalar/scalar_tensor_tensor passes
    (InstTensorScalarPtr hits the 4x fp16 perf mode; TensorTensor/TensorReduce
    do not) with fused accum_out: sum(M*y), sum(y*log p), sum(y*log(1-p)).
  - Pool/GPSIMD (otherwise idle): mask count sum(M) via tensor_scalar accum.
  - PE: row-pool matmuls; DVE grouped reduce on PSUM finishes the 4x4 pool.
  - Attention moments run in a "fat" layout (4 samples stacked on partitions,
    32 partitions each) so each moment is ONE accum op; per-sample values are
    recovered on the host by summing partition groups. The pooled-label tile
    is rotated into that layout through a DRAM bounce.
  - DMA issue is spread over SP and PE sequencers (the issuing SEQ is held
    until HWDGE accepts the copy), never ACT/DVE.
"""

import numpy as np

import concourse.bass as bass
import concourse.bacc as bacc
import concourse.tile as tile
from concourse import mybir
from concourse.bass_utils import run_bass_kernel_spmd

N_CORES = 8
N = 32
S_PER_CORE = N // N_CORES  # 4 samples per core
H = W = 512
P = 128          # SBUF partitions
T = H // P       # 4 row blocks per sample
FD = T * W       # 2048 free elements per partition per sample
N2 = H // 4      # 128 pooled size
K = N2 * N2      # 16384 elements per attention map
PPS = P // S_PER_CORE  # 32 partitions per sample in the fat layout

F32 = mybir.dt.float32
F16 = mybir.dt.float16

# stats tile columns [128, 32] fp32
C_LQ = 0          # +s     : sum(log(1-p)) per sample (ACT accum)
C_SQ = 4          # +3s+{0,1,2}: sum(M*y), sum(y*log p), sum(y*log(1-p))
C_M = 16          # +s     : sum(M) per sample (Pool accum)
C_V, C_H, C_VH, C_V2, C_H2, C_VL, C_HL, C_VHL, C_L2, C_L = range(20, 30)
STATS_W = 32


def _build_nc():
    nc = bacc.Bacc(trn_type="TRN2")

    d_logits = nc.dram_tensor("logits", [S_PER_CORE, P, FD], F16,
                              kind="ExternalInput")
    d_labels = nc.dram_tensor("labels", [S_PER_CORE, P, FD], F16,
                              kind="ExternalInput")
    d_att = nc.dram_tensor("att", [P, 2 * S_PER_CORE * N2], F16,
                           kind="ExternalInput")
    d_pool = nc.dram_tensor("poolmat", [P, T * P], F16, kind="ExternalInput")

    d_lpool = nc.dram_tensor("lpool", [S_PER_CORE, PPS, T, N2], F16,
                             kind="Internal")
    d_stats = nc.dram_tensor("stats", [P, STATS_W], F32,
                             kind="ExternalOutput")
    d_stats_act = nc.dram_tensor("stats_act", [P, S_PER_CORE], F32,
                                 kind="ExternalOutput")

    with tile.TileContext(nc) as tc:
        with (
            tc.tile_pool(name="consts", bufs=1) as consts,
            tc.tile_pool(name="big", bufs=2) as big,
            tc.tile_pool(name="junk", bufs=1) as junkp,
            tc.tile_pool(name="psum", bufs=2, space="PSUM") as psump,
        ):
            stats = consts.tile([P, STATS_W], F32)
            stats_act = consts.tile([P, S_PER_CORE], F32)
            attm = consts.tile([P, 2, S_PER_CORE * N2], F16)
            poolm = consts.tile([P, T, P], F16)
            lpool = consts.tile([P, S_PER_CORE, N2], F16)
            lfat = consts.tile([P, S_PER_CORE * N2], F16)
            vh = consts.tile([P, S_PER_CORE * N2], F16)
            junkf = consts.tile([P, S_PER_CORE * N2], F16)
            junk1 = junkp.tile([P, FD], F16, tag="junk1")
            junk2 = junkp.tile([P, FD], F16, tag="junk2")
            junkg = junkp.tile([P, FD], F16, tag="junkg")

            vt = attm[:, 0, :]
            ht = attm[:, 1, :]

            for s in range(S_PER_CORE):
                pt = big.tile([P, FD], F16, tag="p")
                nc.sync.dma_start(out=pt, in_=d_logits[s])
                yt = big.tile([P, T, W], F16, tag="y")
                nc.sync.dma_start(
                    out=yt, in_=d_labels[s].rearrange("p (t w) -> p t w", t=T))
                if s == 0:
                    nc.scalar.dma_start(out=attm,
                                        in_=d_att.rearrange("p (q f) -> p q f",
                                                            q=2))
                    nc.scalar.dma_start(
                        out=poolm, in_=d_pool.rearrange("p (t m) -> p t m",
                                                        t=T))
                ytf = yt.rearrange("p t w -> p (t w)")

                # ACT: logs (bottleneck engine - nothing else runs here)
                lp = big.tile([P, FD], F16, tag="lp")
                nc.scalar.activation(
                    out=lp, in_=pt, func=mybir.ActivationFunctionType.Ln)
                lq = big.tile([P, FD], F16, tag="lq")
                nc.scalar.activation(
                    out=lq, in_=pt, func=mybir.ActivationFunctionType.Ln,
                    scale=-1.0, bias=1.0,
                    accum_out=stats_act[:, s:s + 1])

                # DVE: fused dot products (4x bf16 perf mode + accum)
                c = C_SQ + 3 * s
                nc.vector.scalar_tensor_tensor(
                    out=junk1, in0=pt, scalar=0.4, in1=ytf,
                    op0=mybir.AluOpType.is_gt, op1=mybir.AluOpType.mult,
                    accum_out=stats[:, c:c + 1])
                nc.vector.scalar_tensor_tensor(
                    out=junk2, in0=lp, scalar=1.0, in1=ytf,
                    op0=mybir.AluOpType.mult, op1=mybir.AluOpType.mult,
                    accum_out=stats[:, c + 1:c + 2])
                nc.vector.scalar_tensor_tensor(
                    out=junk1, in0=lq, scalar=1.0, in1=ytf,
                    op0=mybir.AluOpType.mult, op1=mybir.AluOpType.mult,
                    accum_out=stats[:, c + 2:c + 3])

                # mask count (GPSIMD rejects TensorScalarPtr; DVE has room)
                nc.vector.tensor_scalar(
                    out=junkg, in0=pt, scalar1=0.4, scalar2=None,
                    op0=mybir.AluOpType.is_gt, op1=mybir.AluOpType.add,
                    accum_out=stats[:, C_M + s:C_M + s + 1])

                # PE row-pool matmuls -> PSUM [128, 512]
                ps_pool = psump.tile([P, W], F32, tag="pool")
                for t in range(T):
                    nc.tensor.matmul(
                        ps_pool, lhsT=poolm[:, t, :], rhs=yt[:, t, :],
                        start=(t == 0), stop=(t == T - 1))
                # DVE: finish 4x4 pooling (column groups of 4). bf16 out is
                # safe: 4-term sums, and every consumer tolerates 0.4% noise.
                with nc.allow_low_precision(reason="4-term pooled sums"):
                    nc.vector.tensor_reduce(
                        out=lpool[:, s, :],
                        in_=ps_pool.rearrange("p (g f) -> p g f", f=4),
                        axis=mybir.AxisListType.X, op=mybir.AluOpType.add)

                if s == 0:
                    # attention moments that need only v,h (early, on DVE)
                    nc.vector.scalar_tensor_tensor(
                        out=vh, in0=vt, scalar=1.0, in1=ht,
                        op0=mybir.AluOpType.mult, op1=mybir.AluOpType.mult,
                        accum_out=stats[:, C_VH:C_VH + 1])
                    nc.vector.scalar_tensor_tensor(
                        out=junkf, in0=vt, scalar=1.0, in1=vt,
                        op0=mybir.AluOpType.mult, op1=mybir.AluOpType.mult,
                        accum_out=stats[:, C_V2:C_V2 + 1])
                    nc.vector.scalar_tensor_tensor(
                        out=junkf, in0=ht, scalar=1.0, in1=ht,
                        op0=mybir.AluOpType.mult, op1=mybir.AluOpType.mult,
                        accum_out=stats[:, C_H2:C_H2 + 1])
                    nc.vector.tensor_scalar(
                        out=junkf, in0=vt, scalar1=1.0, scalar2=None,
                        op0=mybir.AluOpType.mult, op1=mybir.AluOpType.add,
                        accum_out=stats[:, C_V:C_V + 1])
                    nc.vector.tensor_scalar(
                        out=junkf, in0=ht, scalar1=1.0, scalar2=None,
                        op0=mybir.AluOpType.mult, op1=mybir.AluOpType.add,
                        accum_out=stats[:, C_H:C_H + 1])

            # rotate pooled labels into the fat layout via a DRAM bounce
            nc.scalar.dma_start(
                out=d_lpool.rearrange("s a b m -> (a b) s m"), in_=lpool)
            nc.scalar.dma_start(
                out=lfat, in_=d_lpool.rearrange("s a b m -> (s a) (b m)"))

            # attention moments involving l (tail, on DVE)
            nc.vector.scalar_tensor_tensor(
                out=junkf, in0=vt, scalar=1.0, in1=lfat,
                op0=mybir.AluOpType.mult, op1=mybir.AluOpType.mult,
                accum_out=stats[:, C_VL:C_VL + 1])
            nc.vector.scalar_tensor_tensor(
                out=junkf, in0=ht, scalar=1.0, in1=lfat,
                op0=mybir.AluOpType.mult, op1=mybir.AluOpType.mult,
                accum_out=stats[:, C_HL:C_HL + 1])
            nc.vector.scalar_tensor_tensor(
                out=junkf, in0=vh, scalar=1.0, in1=lfat,
                op0=mybir.AluOpType.mult, op1=mybir.AluOpType.mult,
                accum_out=stats[:, C_VHL:C_VHL + 1])
            nc.vector.scalar_tensor_tensor(
                out=junkf, in0=lfat, scalar=1.0, in1=lfat,
                op0=mybir.AluOpType.mult, op1=mybir.AluOpType.mult,
                accum_out=stats[:, C_L2:C_L2 + 1])
            nc.vector.tensor_scalar(
                out=junkf, in0=lfat, scalar1=1.0, scalar2=None,
                op0=mybir.AluOpType.mult, op1=mybir.AluOpType.add,
                accum_out=stats[:, C_L:C_L + 1])

            nc.sync.dma_start(out=d_stats[:, :], in_=stats)
            nc.sync.dma_start(out=d_stats_act[:, :], in_=stats_act)

    nc.compile()
    return nc


_NC_CACHE = None


def _get_nc():
    global _NC_CACHE
    if _NC_CACHE is None:
        _NC_CACHE = _build_nc()
    return _NC_CACHE


def _host_combine(stats_all, stats_act):
    """stats_all: [N_CORES, P, STATS_W] float64 -> scalar loss (float32)."""
    smooth = 1.0
    bce_sum = 0.0
    dice_sum = 0.0
    cor_sum = 0.0
    for i in range(N_CORES):
        st = stats_all[i]
        for s in range(S_PER_CORE):
            lq_sum = stats_act[i, :, s].sum()
            c = C_SQ + 3 * s
            my = st[:, c].sum()
            d1 = st[:, c + 1].sum()
            d2 = st[:, c + 2].sum()
            m_cnt = st[:, C_M + s].sum()
            part = slice(PPS * s, PPS * (s + 1))
            sv = st[part, C_V].sum()
            sh = st[part, C_H].sum()
            svh = st[part, C_VH].sum()
            sv2 = st[part, C_V2].sum()
            sh2 = st[part, C_H2].sum()
            svl = st[part, C_VL].sum()
            shl = st[part, C_HL].sum()
            svhl = st[part, C_VHL].sum()
            sl2 = st[part, C_L2].sum()
            sl = st[part, C_L].sum()

            bce_sum += d1 + lq_sum - d2
            dice_sum += 2.0 * (my + smooth) / (m_cnt + sl + smooth)

            mv, mh, ml = sv / K, sh / K, sl / K
            num = svhl - mv * shl - mh * svl - ml * svh + 2.0 * K * mv * mh * ml
            den = np.sqrt((sv2 - K * mv * mv) * (sh2 - K * mh * mh)
                          * (sl2 - K * ml * ml))
            cor_sum += num / den

    bceloss = -bce_sum / (N * H * W)
    diceloss = 1.0 - dice_sum / N
    cor_loss = -cor_sum / N
    return np.float32(0.2 * bceloss + 0.3 * diceloss + 0.5 * cor_loss)


def _make_in_maps(logits, labels, v_attention, h_attention):
    f16 = np.float16

    # clamp AFTER bf16 rounding so Ln(1-p) never sees exactly 1.0
    pmax = np.float16(1.0 - 2.0 ** -11)
    lg = np.minimum(np.asarray(logits, np.float32).astype(f16), pmax)
    # square layout: row r = 128*t + p  ->  partition p, free t*512+w
    lg = np.ascontiguousarray(
        lg.reshape(N, T, P, W).transpose(0, 2, 1, 3).reshape(N, P, FD))
    lb = np.asarray(labels, np.float32).astype(f16)
    lb = np.ascontiguousarray(
        lb.reshape(N, T, P, W).transpose(0, 2, 1, 3).reshape(N, P, FD))

    # fat attention layout: partition 32*s + a holds rows [4a, 4a+4)
    va = np.asarray(v_attention, np.float32).astype(f16).reshape(N, N2, N2)
    ha = np.asarray(h_attention, np.float32).astype(f16).reshape(N, N2, N2)

    # poolm[p, t, m] = 1 iff m == 32*t + p//4 (row-pool chunk t, offset 32t)
    poolm = np.zeros((P, T, P), dtype=np.float32)
    for t in range(T):
        poolm[np.arange(P), t, 32 * t + np.arange(P) // 4] = 1.0
    poolm = poolm.reshape(P, T * P).astype(f16)

    in_maps = []
    for i in range(N_CORES):
        sl = slice(i * S_PER_CORE, (i + 1) * S_PER_CORE)
        att = np.empty((P, 2, S_PER_CORE * N2), dtype=f16)
        # att[32s+a, q, :] = {v,h}[4i+s, 4a:4a+4, :] flattened
        att[:, 0, :] = va[sl].reshape(S_PER_CORE * PPS, T * N2)
        att[:, 1, :] = ha[sl].reshape(S_PER_CORE * PPS, T * N2)
        att = np.ascontiguousarray(att.reshape(P, 2 * S_PER_CORE * N2))
        in_maps.append({
            "logits": lg[sl],
            "labels": lb[sl],
            "att": att,
            "poolmat": poolm,
        })
    return in_maps


def kernel(logits, labels, v_attention, h_attention):
    nc = _get_nc()
    in_maps = _make_in_maps(logits, labels, v_attention, h_attention)
    res = run_bass_kernel_spmd(nc, in_maps, core_ids=list(range(N_CORES)))
    stats_all = np.stack(
        [r["stats"].astype(np.float64) for r in res.results], axis=0)
    stats_act = np.stack(
        [r["stats_act"].astype(np.float64) for r in res.results], axis=0)
    return _host_combine(stats_all, stats_act)
